# revision 25
# baseline (speedup 1.0000x reference)
"""Flat paged-attention (vLLM flat_pa, GQA, const-normalized softmax) on 8 TRN2 cores.

Sharding: data-parallel over decode sequences. Core c owns sequences
[8c, 8c+8) = 256 fetched blocks. The host gathers each core's K/V blocks
from the caches (the block_list indirection), converts to bf16, and lays
them out chunk-major (one chunk = 2 sequences of one kv head = one fully
contiguous HBM slab) so every device DMA reads HBM sequentially:

  kt[c, d, (s2,n,p)]  -- K gathered + transposed so head-dim is the SBUF
                         partition axis (QK^T contracts over d), bf16
  vt[c, p, (s2,n,d+1)] -- V gathered, pos on partitions (PV contracts over
                         pos), bf16. Column d==HD holds exp(block_bias)
                         (the softmax-denominator "ones" column), and all
                         HD value columns are pre-scaled by exp(block_bias)
                         so masked slots contribute exactly zero to both
                         numerator and denominator -- no bias work on device.
  qt[d, (h,s,q)]      -- queries, scale baked in, bf16

K and V stream on the two independent HWDGE DMA rings (SP / ACT engines,
~190 GB/s each).
Chunk-level software pipeline: per chunk, 64 K-stationary matmuls give
scores^T [pos, 64*4q] in one PSUM tile, one ACT op computes
exp(x - CONST_VAL) into bf16 SBUF, then 64 accumulating PV matmuls (for
the previous chunk, hiding the exp latency from the PE) give per-seq
output [4, HD+1] whose last column is the group softmax denominator.
Division by the per-sequence denominator happens once per (head, seq)
(valid because the const-normalized softmax denominator is shared by all
blocks of a sequence).
"""

import sys

sys.path.insert(0, "/opt/trn_rl_repo")

import numpy as np
import ml_dtypes

BF16 = ml_dtypes.bfloat16

B = 64
BPS = 32           # blocks per sequence
BS = 128           # block size (tokens)
KVH = 8
QPK = 4            # q heads per kv head
HD = 128
NCORES = 8
SPC = 8            # sequences per core
BPC = SPC * BPS    # 256 blocks per core
CONST_VAL = 10.0
SCALE = 1.0 / np.sqrt(HD)
CH = 2 * BPS * BS          # K cols per chunk (2 sequences)
CHV = 2 * BPS * (HD + 1)   # V cols per chunk incl. denominator column
NCH = KVH * (SPC // 2)     # chunks per core

_NC_CACHE = {}


def build_nc(reps=1, variant="full"):
    """Build + compile the per-core Bass program. reps>1 wraps the body in a
    dynamic For_i loop (used only for timing). variant: "full" (the real
    kernel), "dmaonly" (just the HBM streams), "computeonly" (one chunk
    DMA, full compute against it) — the latter two only for perf triage."""
    key = (reps, variant)
    if key in _NC_CACHE:
        return _NC_CACHE[key]
    from concourse import bacc, mybir
    import concourse.tile as tile

    f32 = mybir.dt.float32
    bf16 = mybir.dt.bfloat16
    nc = bacc.Bacc("TRN2", target_bir_lowering=False, debug=False, num_devices=NCORES)

    # chunk-major layouts: each chunk's [128, CH] slab is one fully
    # contiguous HBM region, so every DMA reads HBM sequentially.
    kt = nc.dram_tensor("kt", [NCH, HD, CH], bf16, kind="ExternalInput")
    vt = nc.dram_tensor("vt", [NCH, BS, CHV], bf16, kind="ExternalInput")
    qt = nc.dram_tensor("qt", [HD, KVH * SPC * QPK], bf16, kind="ExternalInput")
    out = nc.dram_tensor("out", [QPK, KVH * SPC * HD], f32, kind="ExternalOutput")

    with tile.TileContext(nc) as tc:
        from contextlib import ExitStack

        with ExitStack() as ctx:
            cpool = ctx.enter_context(tc.tile_pool(name="const", bufs=1))
            kpool = ctx.enter_context(tc.tile_pool(name="k", bufs=4))
            vpool = ctx.enter_context(tc.tile_pool(name="v", bufs=4))
            ppool = ctx.enter_context(tc.tile_pool(name="p", bufs=3))
            rpool = ctx.enter_context(tc.tile_pool(name="r", bufs=2))
            opool = ctx.enter_context(tc.tile_pool(name="osb", bufs=1))
            qkps = ctx.enter_context(tc.tile_pool(name="qkps", bufs=2, space="PSUM"))
            ops = ctx.enter_context(tc.tile_pool(name="ops", bufs=2, space="PSUM"))

            qt_sb = cpool.tile([HD, KVH * SPC * QPK], bf16)
            nc.sync.dma_start(out=qt_sb[:], in_=qt[:])
            negc = cpool.tile([BS, 1], f32)
            nc.gpsimd.memset(negc[:], -CONST_VAL)
            out_sb = opool.tile([QPK, KVH * SPC * HD], f32)
            if variant == "dmaonly":
                nc.gpsimd.memset(out_sb[:], 0.0)

            if variant == "computeonly":
                kch0 = cpool.tile([HD, CH], bf16)
                nc.sync.dma_start(out=kch0[:], in_=kt[0])
                vch0 = cpool.tile([BS, CHV], bf16)
                nc.sync.dma_start(out=vch0[:], in_=vt[0])

            NCHUNK = KVH * (SPC // 2)

            def body():
                # Chunk-level software pipeline (one chunk = 2 sequences of
                # one kv head). Per chunk c: DMA-issue(c+1) -> QK(c)
                # [64 matmuls into one PSUM tile] -> one exp(c) -> PV(c-1)
                # [64 matmuls]. The 1-chunk lag means PV never stalls the PE
                # waiting on the same chunk's exp, and the ACT engine stream
                # alternates one V-DMA issue with one exp.
                chunks = {}   # chunk index -> (kch, vch)
                pend = [None]  # (pe, vch, hs0) for chunk c-1

                # One HWDGE ring tops out around ~190 GB/s (contiguous
                # source); K and V stream on the two independent HWDGE
                # rings (SP and ACT engines).
                def issue_chunk(c):
                    if c >= NCHUNK:
                        return
                    if variant == "computeonly":
                        chunks[c] = (kch0, vch0)
                        return
                    kch = kpool.tile([HD, CH], bf16)
                    nc.sync.dma_start(out=kch[:], in_=kt[c])
                    vch = vpool.tile([BS, CHV], bf16)
                    nc.scalar.dma_start(out=vch[:], in_=vt[c])
                    chunks[c] = (kch, vch)

                def emit_pv(p):
                    pe_t, vch_t, hs0 = p
                    for sl in range(2):
                        o_ps = ops.tile([QPK, HD + 1], f32)
                        for nl in range(BPS):
                            b = sl * BPS + nl
                            nc.tensor.matmul(
                                out=o_ps[:],
                                lhsT=pe_t[:, b * QPK:(b + 1) * QPK],
                                rhs=vch_t[:, b * (HD + 1):(b + 1) * (HD + 1)],
                                start=(nl == 0),
                                stop=(nl == BPS - 1),
                            )
                        rec = rpool.tile([QPK, 1], f32)
                        nc.vector.reciprocal(rec[:], o_ps[:, HD:HD + 1])
                        nc.vector.tensor_scalar_mul(
                            out_sb[:, (hs0 + sl) * HD:(hs0 + sl + 1) * HD],
                            o_ps[:, 0:HD],
                            rec[:],
                        )

                issue_chunk(0)
                for c in range(NCHUNK):
                    issue_chunk(c + 1)
                    if variant == "dmaonly":
                        continue
                    kch, vch = chunks[c]
                    h, sp = divmod(c, SPC // 2)
                    hs0 = h * SPC + sp * 2   # first (head, seq) out column
                    qk = qkps.tile([BS, 2 * BPS * QPK], f32)
                    for sl in range(2):
                        qcol = (hs0 + sl) * QPK
                        for nl in range(BPS):
                            b = sl * BPS + nl
                            nc.tensor.matmul(
                                out=qk[:, b * QPK:(b + 1) * QPK],
                                lhsT=kch[:, b * BS:(b + 1) * BS],
                                rhs=qt_sb[:, qcol:qcol + QPK],
                                start=True,
                                stop=True,
                            )
                    pe = ppool.tile([BS, 2 * BPS * QPK], bf16, tag="pe")
                    if variant == "fullnoexp":
                        # perf-triage only (wrong values): PSUM->SBUF copy on
                        # DVE so the ACT engine does nothing but DMA issue.
                        nc.vector.tensor_copy(pe[:], qk[:])
                    else:
                        nc.scalar.activation(
                            pe[:], qk[:], mybir.ActivationFunctionType.Exp,
                            bias=negc[:],
                        )
                    if pend[0] is not None:
                        emit_pv(pend[0])
                    pend[0] = (pe, vch, hs0)
                if pend[0] is not None:
                    emit_pv(pend[0])
                    pend[0] = None
                nc.sync.dma_start(out=out[:], in_=out_sb[:])

            if reps == 1:
                body()
            else:
                with tc.For_i(0, reps, 1):
                    body()

    nc.compile()
    _NC_CACHE[key] = nc
    return nc


def prep_inputs(query, key_cache, value_cache, block_list, block_mapping,
                block_bias, block_groups):
    """Host-side shard + gather + layout + bf16 conversion. Returns per-core
    in_maps."""
    query = np.asarray(query, dtype=np.float32)
    key_cache = np.asarray(key_cache, dtype=np.float32)
    value_cache = np.asarray(value_cache, dtype=np.float32)
    block_list = np.asarray(block_list)
    block_bias = np.asarray(block_bias, dtype=np.float32)
    block_groups = np.asarray(block_groups)

    # per-sequence fetched-block rows (pad to BPS with masked dummies)
    seq_rows = np.zeros((B, BPS), dtype=np.int64)
    pad_mask = np.zeros((B, BPS), dtype=bool)
    for s in range(B):
        rows = np.flatnonzero(block_groups == s)
        assert len(rows) <= BPS, f"sequence {s} has {len(rows)} > {BPS} blocks"
        seq_rows[s, :len(rows)] = rows
        pad_mask[s, len(rows):] = True

    qs = (query.reshape(B, KVH, QPK, HD) * SCALE)  # (s, h, q, d)

    in_maps = []
    for c in range(NCORES):
        rows = seq_rows[c * SPC:(c + 1) * SPC].reshape(-1)          # [256]
        pmask = pad_mask[c * SPC:(c + 1) * SPC].reshape(-1)         # [256]
        bl = block_list[rows].astype(np.int64)
        gk = key_cache[bl]                                           # [256,p,h,d]
        # chunk-major: [NCH, HD, CH], chunk c=(h,sp) contiguous in HBM
        kt_c = np.ascontiguousarray(
            gk.transpose(2, 3, 0, 1)                 # [h, d, n, p]
            .reshape(KVH, HD, SPC // 2, CH)
            .transpose(0, 2, 1, 3)                   # [h, sp, d, CH]
            .astype(BF16)).reshape(NCH, HD, CH)
        # exp(bias) mask: 1 for live slots, 0 for masked/padded slots
        # (exact for bias in {0, -30000}); scales V and forms the
        # denominator column, so masked slots contribute exactly 0.
        m = np.exp(block_bias[rows])                                 # [256, p]
        m[pmask] = 0.0
        gv = value_cache[bl] * m[:, :, None, None]                   # [256,p,h,d]
        gv = np.concatenate(
            [gv, np.broadcast_to(m[:, :, None, None], (BPC, BS, KVH, 1))],
            axis=3)
        vt_c = np.ascontiguousarray(
            gv.transpose(2, 1, 0, 3)                 # [h, p, n, d+1]
            .reshape(KVH, BS, SPC // 2, CHV)
            .transpose(0, 2, 1, 3)                   # [h, sp, p, CHV]
            .astype(BF16)).reshape(NCH, BS, CHV)
        # queries for this core: (d, h, s, q)
        qt_c = np.ascontiguousarray(
            qs[c * SPC:(c + 1) * SPC].transpose(3, 1, 0, 2).astype(BF16)
        ).reshape(HD, -1)
        in_maps.append({"kt": kt_c, "vt": vt_c, "qt": qt_c})
    return in_maps


def assemble_output(results):
    out = np.zeros((B, KVH * QPK, HD), dtype=np.float32)
    for c in range(NCORES):
        o = results[c]["out"].reshape(QPK, KVH, SPC, HD)  # (q,h,s,d)
        out[c * SPC:(c + 1) * SPC] = o.transpose(2, 1, 0, 3).reshape(SPC, KVH * QPK, HD)
    return out


def kernel(query, key_cache, value_cache, block_list, block_mapping,
           block_bias, block_groups):
    from concourse.bass_utils import run_bass_kernel_spmd

    nc = build_nc(reps=1)
    in_maps = prep_inputs(query, key_cache, value_cache, block_list,
                          block_mapping, block_bias, block_groups)
    res = run_bass_kernel_spmd(nc, in_maps, core_ids=list(range(NCORES)))
    return assemble_output(res.results)
